# revision 25
# baseline (speedup 1.0000x reference)
"""Distributed Trainium2 kernel for nn_AttentionCircuit (routed low-rank QKV + causal attention).

Sharding: 8 cores = 4 batches x 2 token-halves. Each core computes the routed
projections for its 1024 tokens; K^T (d-major) and V (token-major) are
exchanged within the batch pair via two split 2-rank AllGathers, each issued
as soon as its projection finishes so the transfer hides under later matmuls.
Each core then runs causal attention for all 16 heads over its own 1024
queries against all 2048 keys, two heads at a time concurrently in the PE
array via row-tiling (contraction=64 per head at tile_position (0,0)/(64,0)).
W_O is applied locally.

Softmax uses no running max: scores are bounded on this data (|s| ~ 25), so
f32 exp is safe and normalization cancels. The softmax denominator rides the
PV matmul as a ones-column appended to V (M=65); all normalizations are
applied in one batched pass before W_O.
"""

import numpy as np
import ml_dtypes

DEBUG_DUMPS = False
B, S, D = 4, 2048, 1024
R = 64
NB = 32            # neurons per routing bank
H = 16             # heads
DH = D // H        # 64
T = S // 2         # tokens per core = 1024
NCORES = 8

BF16 = ml_dtypes.bfloat16


def _build_graph():
    import concourse.mybir as mybir
    import concourse.tile as tile
    from concourse import bacc
    from concourse.bass import AP
    from concourse.masks import make_identity

    fp32 = mybir.dt.float32
    bf16 = mybir.dt.bfloat16
    ALU = mybir.AluOpType
    ACTF = mybir.ActivationFunctionType

    nc = bacc.Bacc(None, target_bir_lowering=False, num_devices=NCORES)

    xT_p = nc.declare_dram_parameter("xT", [D, T], bf16, isOutput=False)
    F_p = nc.declare_dram_parameter("F", [D, 2 * NB * R], bf16, isOutput=False)      # [d, (n r)]
    Wr_p = nc.declare_dram_parameter("Wrep", [T, 2 * NB * R], bf16, isOutput=False)  # w repeated over r
    Rc_p = nc.declare_dram_parameter("Rcat", [2 * NB * R, D], bf16, isOutput=False)  # [(n r), d]
    WOT_p = nc.declare_dram_parameter("WOT", [D, D], bf16, isOutput=False)           # W_O.T
    wqt_p = nc.declare_dram_parameter("wqt", [NB, T], bf16, isOutput=False)
    wkt_p = nc.declare_dram_parameter("wkt", [NB, T], bf16, isOutput=False)
    wvt_p = nc.declare_dram_parameter("wvt", [NB, T], bf16, isOutput=False)
    A_p = nc.declare_dram_parameter("A", [128, 512], fp32, isOutput=False)           # A[kk,j] = kk - j
    ct_p = nc.declare_dram_parameter("ct", [128, 32], fp32, isOutput=False)          # per (qb,kt) threshold
    out_p = nc.declare_dram_parameter("out", [T, D], fp32, isOutput=True)
    if DEBUG_DUMPS:
        dbg_h = nc.declare_dram_parameter("dbg_h", [64, T], fp32, isOutput=True)
        dbg_k = nc.declare_dram_parameter("dbg_k", [D, T], fp32, isOutput=True)
        dbg_v = nc.declare_dram_parameter("dbg_v", [T, D], fp32, isOutput=True)
        dbg_q = nc.declare_dram_parameter("dbg_q", [128, T], fp32, isOutput=True)
        dbg_ao = nc.declare_dram_parameter("dbg_ao", [128, T], fp32, isOutput=True)
        dbg_l = nc.declare_dram_parameter("dbg_l", [16, T], fp32, isOutput=True)

    groups = [[0, 1], [2, 3], [4, 5], [6, 7]]
    NT = T // 128
    ND = D // 128
    NKTQ = [16, 8]      # kt loop bound per q-block slot (balanced causal split)

    with tile.TileContext(nc) as tc:
        with (
            tc.tile_pool(name="w", bufs=1) as wpool,
            tc.tile_pool(name="big", bufs=1) as big,
            tc.tile_pool(name="hwa", bufs=1) as hwa,
            tc.tile_pool(name="hwb", bufs=1) as hwb,
            tc.tile_pool(name="stage", bufs=1) as stg,
            tc.tile_pool(name="mm", bufs=3, space="PSUM") as pmm,
            tc.tile_pool(name="pop", bufs=2, space="PSUM") as ppo,
            tc.tile_pool(name="dram", bufs=1, space="DRAM") as dram,
        ):
            # ---------------- persistent small inputs ----------------
            ident = wpool.tile([128, 128], bf16, tag="idb")
            make_identity(nc, ident[:, :])

            wqt_sb = wpool.tile([NB, T], bf16, tag="wqt")
            wkt_sb = wpool.tile([NB, T], bf16, tag="wkt")
            wvt_sb = wpool.tile([NB, T], bf16, tag="wvt")
            nc.sync.dma_start(out=wqt_sb[:, :], in_=wqt_p[:, :])
            nc.sync.dma_start(out=wkt_sb[:, :], in_=wkt_p[:, :])
            nc.sync.dma_start(out=wvt_sb[:, :], in_=wvt_p[:, :])
            A_sb = wpool.tile([128, 512], fp32, tag="A")
            nc.sync.dma_start(out=A_sb[:, :], in_=A_p[:, :])
            ct_sb = wpool.tile([128, 32], fp32, tag="ct")
            nc.sync.dma_start(out=ct_sb[:, :], in_=ct_p[:, :])

            hT_sb = [wpool.tile([64, T], bf16, tag=f"hT{b}", name=f"hT{b}") for b in range(2)]
            Lt = wpool.tile([16, T], bf16, tag="Lt")
            Li = wpool.tile([16, T], bf16, tag="Li")
            ones_t = wpool.tile([128, 1], bf16, tag="ones")
            nc.gpsimd.memset(ones_t[:, :], 1.0)
            kloc = wpool.tile([128, T], bf16, tag="kloc")
            hstore = wpool.tile([128, NT * 64], bf16, tag="hstore")

            xT_sb = [big.tile([128, T], bf16, tag=f"xT{dt}", name=f"xT{dt}") for dt in range(ND)]
            for dt in range(ND):
                nc.sync.dma_start(out=xT_sb[dt][:, :], in_=xT_p[dt * 128:(dt + 1) * 128, :])
            FB = [big.tile([128, 1024], bf16, tag=f"FB{i}", name=f"FB{i}") for i in range(32)]

            def load_F(cb):
                for dt in range(ND):
                    nc.sync.dma_start(out=FB[dt * 4 + cb][:, :], in_=F_p[dt * 128:(dt + 1) * 128, cb * 1024:(cb + 1) * 1024])

            load_F(0)

            # ---------------- stage 1 ----------------
            hhalf = [None]

            def stage1_cb(cb):
                bank, half = cb // 2, cb % 2
                if cb < 3:
                    load_F(cb + 1)
                for tt in range(NT):
                    wt = stg.tile([128, 1024], bf16, tag="wt", name=f"wt{cb}_{tt}", bufs=2)
                    nc.sync.dma_start(out=wt[:, :], in_=Wr_p[tt * 128:(tt + 1) * 128, cb * 1024:(cb + 1) * 1024])
                    ps = pmm.tile([128, 1024], fp32, tag="mm", name="ps1")
                    for dt in range(ND):
                        for nb2 in range(2):
                            nc.tensor.matmul(
                                ps[:, nb2 * 512:(nb2 + 1) * 512],
                                xT_sb[dt][:, tt * 128:(tt + 1) * 128],
                                FB[dt * 4 + cb][:, nb2 * 512:(nb2 + 1) * 512],
                                start=(dt == 0),
                                stop=(dt == ND - 1),
                            )
                    nc.vector.tensor_tensor(out=wt[:, :], in0=ps[:, :], in1=wt[:, :], op=ALU.mult)
                    if half == 0:
                        hh = hstore[:, tt * 64:(tt + 1) * 64]
                    else:
                        hh = stg.tile([128, 64], bf16, tag="hh1", name=f"hh{cb}_{tt}", bufs=2)[:, :]
                    for w2 in (512, 256, 128):
                        nc.vector.tensor_tensor(out=wt[:, 0:w2], in0=wt[:, 0:w2], in1=wt[:, w2:2 * w2], op=ALU.add)
                    nc.vector.tensor_tensor(out=hh, in0=wt[:, 0:64], in1=wt[:, 64:128], op=ALU.add)
                    if half == 1:
                        hf = stg.tile([128, 64], bf16, tag="hf", name=f"hf{bank}_{tt}", bufs=2)
                        nc.vector.tensor_tensor(out=hf[:, :], in0=hstore[:, tt * 64:(tt + 1) * 64], in1=hh, op=ALU.add)
                        pt = ppo.tile([64, 128], bf16, tag="po", name="pt1")
                        nc.tensor.transpose(pt[:, :], hf[:, :], ident[:, :])
                        nc.scalar.copy(out=hT_sb[bank][:, tt * 128:(tt + 1) * 128], in_=pt[:, :])

            def build_hw(hwt, wsb, hTsrc, tag, ns=range(NB)):
                for n in ns:
                    bc = stg.tile([64, T], bf16, tag="bc", name=f"bc{tag}_{n}", bufs=3)
                    wrow = wsb[n:n + 1, :]
                    nc.sync.dma_start(out=bc[:, :], in_=AP(wrow.tensor, wrow.offset, [[0, 64], [1, T]]))
                    nc.vector.tensor_tensor(
                        out=hwt[n // 2][(n % 2) * 64:(n % 2) * 64 + 64, :],
                        in0=hTsrc[:, :], in1=bc[:, :], op=ALU.mult,
                    )

            stage1_cb(0)
            stage1_cb(1)
            # hT_qk ready -> hw for K overlaps remaining stage-1 matmuls
            hwk = [hwa.tile([128, 1024], bf16, tag=f"hwa{i}", name=f"hwk{i}") for i in range(16)]
            build_hw(hwk, wkt_p, hT_sb[0], "k")
            # R bank rqk: reuses F slots of cb 0/1 (already dead)
            Rk = [big.tile([128, D], bf16, tag=f"FB{(i // 2) * 4 + (i % 2)}", name=f"Rk{i}") for i in range(16)]
            for i in range(16):
                nc.sync.dma_start(out=Rk[i][:, :], in_=Rc_p[i * 128:(i + 1) * 128, :])
            stage1_cb(2)
            stage1_cb(3)
            Rv = [big.tile([128, D], bf16, tag=f"FB{(i // 2) * 4 + 2 + (i % 2)}", name=f"Rv{i}") for i in range(16)]
            for i in range(16):
                nc.sync.dma_start(out=Rv[i][:, :], in_=Rc_p[(16 + i) * 128:(17 + i) * 128, :])

            if DEBUG_DUMPS:
                dtmp = stg.tile([64, T], fp32, tag="nbc", name="dbgh")
                nc.vector.tensor_copy(out=dtmp[:, :], in_=hT_sb[0][:, :])
                nc.sync.dma_start(out=dbg_h[:, :], in_=dtmp[:, :])

            # ---------------- stage 2 ----------------
            send_K = dram.tile([D, T], bf16, tag="sendK")        # K^T [d, own t]
            send_V = dram.tile([T, D], bf16, tag="sendV")        # V   [own t, d]
            recv_K = dram.tile([2 * D, T], bf16, tag="recvK")
            recv_V = dram.tile([S, D], bf16, tag="recvV")

            # K projection (d-major); V hw build interleaved on DVE
            hwv = [hwb.tile([128, 1024], bf16, tag=f"hwb{i}", name=f"hwv{i}") for i in range(8)]
            hwv += [big.tile([128, 1024], bf16, tag=f"xT{i}", name=f"hwv{8 + i}") for i in range(8)]
            for dt in range(ND):
                ps = pmm.tile([128, 1024], fp32, tag="mm", name="ps2k")
                for pair in range(16):
                    for th in range(2):
                        nc.tensor.matmul(
                            ps[:, th * 512:(th + 1) * 512],
                            Rk[pair][:, dt * 128:(dt + 1) * 128],
                            hwk[pair][:, th * 512:(th + 1) * 512],
                            start=(pair == 0), stop=(pair == 15),
                        )
                st = stg.tile([128, 1024], bf16, tag="st", name=f"stk{dt}", bufs=2)
                nc.vector.tensor_copy(out=st[:, :], in_=ps[:, :])
                nc.sync.dma_start(out=send_K[dt * 128:(dt + 1) * 128, :], in_=st[:, :])
                build_hw(hwv, wvt_p, hT_sb[1], "v", ns=range(dt * 4, dt * 4 + 4))
                if DEBUG_DUMPS:
                    dtk = stg.tile([128, 1024], fp32, tag="fo", name=f"dtk{dt}", bufs=2)
                    nc.vector.tensor_copy(out=dtk[:, :], in_=ps[:, :])
                    nc.sync.dma_start(out=dbg_k[dt * 128:(dt + 1) * 128, :], in_=dtk[:, :])
            nc.gpsimd.collective_compute(
                "AllGather", ALU.bypass, replica_groups=groups,
                ins=[send_K[:, :].opt()], outs=[recv_K[:, :].opt()],
            )

            # V projection (token-major); Q hw build interleaved on DVE
            hwq = [hwa.tile([128, 1024], bf16, tag=f"hwa{i}", name=f"hwq{i}") for i in range(16)]
            for tb in range(NT):
                ps = pmm.tile([128, 1024], fp32, tag="mm", name="ps2v")
                for pair in range(16):
                    for dh in range(2):
                        nc.tensor.matmul(
                            ps[:, dh * 512:(dh + 1) * 512],
                            hwv[pair][:, tb * 128:(tb + 1) * 128],
                            Rv[pair][:, dh * 512:(dh + 1) * 512],
                            start=(pair == 0), stop=(pair == 15),
                        )
                st = stg.tile([128, 1024], bf16, tag="st", name=f"stv{tb}", bufs=2)
                nc.vector.tensor_copy(out=st[:, :], in_=ps[:, :])
                nc.sync.dma_start(out=send_V[tb * 128:(tb + 1) * 128, :], in_=st[:, :])
                build_hw(hwq, wqt_p, hT_sb[0], "q", ns=range(tb * 4, tb * 4 + 4))
                if DEBUG_DUMPS:
                    dtv = stg.tile([128, 1024], fp32, tag="fo", name=f"dtv{tb}", bufs=2)
                    nc.vector.tensor_copy(out=dtv[:, :], in_=ps[:, :])
                    nc.sync.dma_start(out=dbg_v[tb * 128:(tb + 1) * 128, :], in_=dtv[:, :])
            nc.gpsimd.collective_compute(
                "AllGather", ALU.bypass, replica_groups=groups,
                ins=[send_V[:, :].opt()], outs=[recv_V[:, :].opt()],
            )

            # Q projection (d-major, stays on-chip)
            QT_sb = []
            for dt in range(ND):
                ps = pmm.tile([128, 1024], fp32, tag="mm", name="ps2q")
                for pair in range(16):
                    for th in range(2):
                        nc.tensor.matmul(
                            ps[:, th * 512:(th + 1) * 512],
                            Rk[pair][:, dt * 128:(dt + 1) * 128],
                            hwq[pair][:, th * 512:(th + 1) * 512],
                            start=(pair == 0), stop=(pair == 15),
                        )
                qt = big.tile([128, 1024], bf16, tag=f"FB{dt * 4 + 2}", name=f"QT{dt}")
                nc.vector.tensor_copy(out=qt[:, :], in_=ps[:, :])
                QT_sb.append(qt)

            WOT_sb = [big.tile([128, D], bf16, tag=f"xT{dt}", name=f"wo{dt}") for dt in range(ND)]
            for dt in range(ND):
                nc.sync.dma_start(out=WOT_sb[dt][:, :], in_=WOT_p[dt * 128:(dt + 1) * 128, :])

            # causal keep masks, shared by all heads: m01[qb][:, kt*512:...] = (A <= ct)
            m01 = []
            for qb, kt0 in ((0, 8), (1, 0)):   # slot0 kt<8 is causal-clean on every core
                nm = NKTQ[qb] - kt0
                m = big.tile([128, nm * 512], bf16, tag=f"FB{7 if qb == 0 else 11}", name=f"m01_{qb}")
                for i in range(nm):
                    nc.vector.tensor_scalar(
                        m[:, i * 512:(i + 1) * 512], A_sb[:, :],
                        ct_sb[:, qb * 16 + kt0 + i: qb * 16 + kt0 + i + 1], None, ALU.is_le,
                    )
                m01.append(m)

            # ---------------- attention ----------------
            AO_sb = [big.tile([128, T], bf16, tag=f"FB{dt * 4}", name=f"AO{dt}") for dt in range(ND)]
            ldram0 = dram.tile([16, T], bf16, tag="ldram0")
            ldram = dram.tile([16, T], bf16, tag="ldram")
            va_tags = [9, 13, 17, 21]
            ka_tags = [[1, 15], [5, 19]]
            qa_tags = [[23, 27], [31, 3]]
            for hp in range(8):
                # diag scores d[q] = Q_q . K_q for this head pair (own tokens)
                nc.sync.dma_start(out=kloc[:, :], in_=send_K[hp * 128:(hp + 1) * 128, :])
                nc.vector.tensor_tensor(out=kloc[:, :], in0=QT_sb[hp][:, :], in1=kloc[:, :], op=ALU.mult)
                ka = []
                qa = []
                for h2 in range(2):
                    hh_row = (2 * hp + h2) * 64
                    k_h = big.tile([65, S], bf16, tag=f"FB{ka_tags[hp % 2][h2]}", name=f"ka{hp}_{h2}")
                    # global keys 0:512=A[512:1024], 512:1024=B[512:1024], 1024:1536=B[0:512], 1536:2048=A[0:512]
                    nc.sync.dma_start(out=k_h[0:64, 0:512], in_=recv_K[hh_row:hh_row + 64, 512:1024])
                    nc.sync.dma_start(out=k_h[0:64, 512:1024], in_=recv_K[D + hh_row:D + hh_row + 64, 512:1024])
                    nc.sync.dma_start(out=k_h[0:64, 1024:1536], in_=recv_K[D + hh_row:D + hh_row + 64, 0:512])
                    nc.sync.dma_start(out=k_h[0:64, 1536:2048], in_=recv_K[hh_row:hh_row + 64, 0:512])
                    nc.gpsimd.memset(k_h[64:65, :], 1.0)
                    ka.append(k_h)
                    q_h = big.tile([65, T], bf16, tag=f"FB{qa_tags[hp % 2][h2]}", name=f"qa{hp}_{h2}")
                    nc.vector.tensor_copy(out=q_h[0:64, :], in_=QT_sb[hp][h2 * 64:(h2 + 1) * 64, :])
                    for qb in range(2):
                        dg = ppo.tile([1, 512], fp32, tag="po", name=f"dg{hp}_{h2}_{qb}")
                        nc.tensor.matmul(
                            dg[:, :], ones_t[h2 * 64:(h2 + 1) * 64, :],
                            kloc[h2 * 64:(h2 + 1) * 64, qb * 512:(qb + 1) * 512],
                            start=True, stop=True,
                        )
                        nc.vector.tensor_scalar(
                            q_h[64:65, qb * 512:(qb + 1) * 512], dg[:, :], -1.0, None, ALU.mult,
                        )
                    qa.append(q_h)
                va = []
                for h2 in range(2):
                    v = big.tile([128, 16, 65], bf16, tag=f"FB{va_tags[(hp % 2) * 2 + h2]}", name=f"va{hp}_{h2}")
                    nc.gpsimd.memset(v[:, :, 64:65], 1.0)
                    hh_col = (2 * hp + h2) * 64
                    for ktg, r0 in enumerate((512, T + 512, T, 0)):   # 512-row source per 4-kt group
                        nc.sync.dma_start(
                            out=v[:, ktg * 4:(ktg + 1) * 4, 0:64],
                            in_=recv_V[r0:r0 + 512, hh_col:hh_col + 64].rearrange("(kt p) c -> p kt c", p=128),
                        )
                    va.append(v)
                for qb in range(2):
                    nkt = NKTQ[qb]
                    poA = ppo.tile([65, 512], fp32, tag="po", name=f"poA{hp}_{qb}")
                    poB = ppo.tile([65, 512], fp32, tag="po", name=f"poB{hp}_{qb}")
                    for kt in range(nkt):
                        ss = pmm.tile([128, 1024], fp32, tag="mm", name="ssc")
                        for h2 in range(2):
                            nc.tensor.matmul(
                                ss[:, h2 * 512:(h2 + 1) * 512],
                                ka[h2][:, kt * 128:(kt + 1) * 128],
                                qa[h2][:, qb * 512:(qb + 1) * 512],
                                start=True, stop=True,
                            )
                        pp = big.tile([128, 1024], bf16, tag=["FB25", "FB29", "pp3"][kt % 3], name=f"pp{hp}_{qb}_{kt}")
                        nc.scalar.activation(pp[:, :], ss[:, :], ACTF.Exp, scale=0.125)
                        if not (qb == 0 and kt < 8):   # slot0 kt<8 is causal-clean on every core
                            mi = kt - 8 if qb == 0 else kt
                            nc.vector.tensor_tensor(
                                out=pp[:, 0:512], in0=pp[:, 0:512],
                                in1=m01[qb][:, mi * 512:(mi + 1) * 512], op=ALU.mult,
                            )
                            nc.vector.tensor_tensor(
                                out=pp[:, 512:1024], in0=pp[:, 512:1024],
                                in1=m01[qb][:, mi * 512:(mi + 1) * 512], op=ALU.mult,
                            )
                        nc.tensor.matmul(
                            poA[:, :], va[0][:, kt:kt + 1, :], pp[:, 0:512],
                            start=(kt == 0), stop=(kt == nkt - 1),
                        )
                        nc.tensor.matmul(
                            poB[:, :], va[1][:, kt:kt + 1, :], pp[:, 512:1024],
                            start=(kt == 0), stop=(kt == nkt - 1),
                        )
                    nc.vector.tensor_copy(out=AO_sb[hp][0:64, qb * 512:(qb + 1) * 512], in_=poA[0:64, :])
                    nc.vector.tensor_copy(out=AO_sb[hp][64:128, qb * 512:(qb + 1) * 512], in_=poB[0:64, :])
                    # DVE writes must start at an aligned partition: stage each
                    # denominator row at partition 0 and DMA it to DRAM.
                    for h2, poX in ((0, poA), (1, poB)):
                        dvec = stg.tile([1, 512], bf16, tag="dvec", name=f"dv{hp}_{qb}_{h2}", bufs=2)
                        nc.vector.tensor_copy(out=dvec[:, :], in_=poX[64:65, :])
                        nc.sync.dma_start(
                            out=ldram0[2 * hp + h2:2 * hp + h2 + 1, qb * 512:(qb + 1) * 512],
                            in_=dvec[:, :],
                        )


            # ---------------- normalize + W_O ----------------
            if DEBUG_DUMPS:
                dq = stg.tile([128, T], fp32, tag="fo", name="dbgq", bufs=2)
                nc.vector.tensor_copy(out=dq[:, :], in_=QT_sb[0][:, :])
                nc.sync.dma_start(out=dbg_q[:, :], in_=dq[:, :])
                dao = stg.tile([128, T], fp32, tag="fo", name="dbgao", bufs=2)
                nc.vector.tensor_copy(out=dao[:, :], in_=AO_sb[0][:, :])
                nc.sync.dma_start(out=dbg_ao[:, :], in_=dao[:, :])
                dlb = stg.tile([16, T], bf16, tag="st", name="dbglb", bufs=2)
                nc.sync.dma_start(out=dlb[:, :], in_=ldram0[:, :])
                dl = stg.tile([16, T], fp32, tag="nbc", name="dbgl")
                nc.vector.tensor_copy(out=dl[:, :], in_=dlb[:, :])
                nc.sync.dma_start(out=dbg_l[:, :], in_=dl[:, :])
            ltt = stg.tile([128, 128], bf16, tag="lt2", name="ltt")
            nc.sync.dma_start(out=ltt[:, :], in_=ldram0[:, :].rearrange("h (b c) -> (h b) c", c=128))
            lit = stg.tile([128, 128], bf16, tag="li2", name="lit")
            with nc.allow_low_precision("bf16 softmax denominators; rel tol 2e-2"):
                nc.vector.reciprocal(lit[:, :], ltt[:, :])
            nc.sync.dma_start(out=ldram[:, :].rearrange("h (b c) -> (h b) c", c=128), in_=lit[:, :])
            for dt in range(ND):
                nbc = stg.tile([128, T], bf16, tag="nbc", name=f"nbc{dt}", bufs=2)
                for h2 in range(2):
                    row = ldram[2 * dt + h2:2 * dt + h2 + 1, :]
                    nc.sync.dma_start(out=nbc[h2 * 64:(h2 + 1) * 64, :], in_=AP(row.tensor, row.offset, [[0, 64], [1, T]]))
                nc.vector.tensor_tensor(out=AO_sb[dt][:, :], in0=AO_sb[dt][:, :], in1=nbc[:, :], op=ALU.mult)
            for tt in range(NT):
                ps = pmm.tile([128, 1024], fp32, tag="mm", name="ps3")
                for dt in range(ND):
                    for eh in range(2):
                        nc.tensor.matmul(
                            ps[:, eh * 512:(eh + 1) * 512],
                            AO_sb[dt][:, tt * 128:(tt + 1) * 128],
                            WOT_sb[dt][:, eh * 512:(eh + 1) * 512],
                            start=(dt == 0), stop=(dt == ND - 1),
                        )
                fo = stg.tile([128, 1024], fp32, tag="fo", name="fo", bufs=2)
                nc.vector.tensor_copy(out=fo[:, :], in_=ps[:, :])
                nc.sync.dma_start(out=out_p[tt * 128:(tt + 1) * 128, :], in_=fo[:, :])

    nc.compile()
    return nc


def _host_inputs(x, fqk_weights, fv_weights, rqk_weights_Q, rqk_weights_K, rv_weights,
                 f_neurons, r_neurons, W_O):
    F = np.ascontiguousarray(f_neurons.transpose(1, 0, 2).reshape(D, 2 * NB * R)).astype(BF16)
    Rcat = np.ascontiguousarray(r_neurons.reshape(2 * NB * R, D)).astype(BF16)
    WOT = np.ascontiguousarray(W_O.T).astype(BF16)
    A = np.ascontiguousarray(
        (np.arange(128)[:, None] - np.arange(512)[None, :]).astype(np.float32))

    in_maps = []
    for c in range(NCORES):
        b, half = c // 2, c % 2
        # balanced causal split: even core owns global q-blocks {3,0}, odd {2,1}
        gblks = (3, 0) if half == 0 else (2, 1)
        tok = np.r_[gblks[0] * 512:(gblks[0] + 1) * 512, gblks[1] * 512:(gblks[1] + 1) * 512]
        ct = np.zeros((128, 32), dtype=np.float32)
        for qb in range(2):
            for kt in range(16):
                # keep iff kglob <= qglob:  kk - j <= g*512 - kt*128
                ct[:, qb * 16 + kt] = gblks[qb] * 512 - kt * 128
        w_cat = np.concatenate([fqk_weights[b, tok, :], fv_weights[b, tok, :]], axis=1)
        in_maps.append({
            "xT": np.ascontiguousarray(x[b, tok, :].T).astype(BF16),
            "F": F,
            "Wrep": np.ascontiguousarray(np.repeat(w_cat, R, axis=1)).astype(BF16),
            "Rcat": Rcat,
            "WOT": WOT,
            "wqt": np.ascontiguousarray(rqk_weights_Q[b, tok, :].T).astype(BF16),
            "wkt": np.ascontiguousarray(rqk_weights_K[b, tok, :].T).astype(BF16),
            "wvt": np.ascontiguousarray(rv_weights[b, tok, :].T).astype(BF16),
            "A": A,
            "ct": ct,
        })
    return in_maps


def kernel(x, fqk_weights, fv_weights, rqk_weights_Q, rqk_weights_K, rv_weights,
           f_neurons, r_neurons, W_O, _trace=False):
    from concourse.bass_utils import run_bass_kernel_spmd

    nc = _build_graph()
    in_maps = _host_inputs(x, fqk_weights, fv_weights, rqk_weights_Q, rqk_weights_K,
                           rv_weights, f_neurons, r_neurons, W_O)
    res = run_bass_kernel_spmd(nc, in_maps, core_ids=list(range(NCORES)), trace=_trace)
    out = np.zeros((B, S, D), dtype=np.float32)
    for c in range(NCORES):
        b, half = c // 2, c % 2
        gblks = (3, 0) if half == 0 else (2, 1)
        r = np.asarray(res.results[c]["out"], dtype=np.float32)
        out[b, gblks[0] * 512:(gblks[0] + 1) * 512, :] = r[0:512]
        out[b, gblks[1] * 512:(gblks[1] + 1) * 512, :] = r[512:1024]
    if _trace:
        return out, res
    return out


if __name__ == "__main__":
    print("smoke build only")
    _build_graph()
    print("graph built OK")
